# revision 6
# baseline (speedup 1.0000x reference)
"""Trainium2 Bass kernel for nn_CombineGraph (GCE-GNN LocalAggregator), v2.

Computation (per batch b):
    h = emb_table[inputs[b]]                         # [L, D]
    e_k[i,j] = leakyrelu(sum_d h[i,d]*h[j,d]*a_k[d]) # 4 edge-type logits
    alpha = softmax_j(select-by-adj(e_k), -9e15 fill)
    out[b] = alpha @ h

Sharding: pure data-parallel over batch B=512 across 8 NeuronCores
(64 batches/core). emb table + a-vectors replicated; no collectives.

v2 changes vs v1 (250us -> target ~65us):
  - bf16 everywhere on the PE path (4x matmul throughput) and on DVE
    elementwise ops (2-4x modes).
  - indirect gathers batched 8 batches per SWDGE instruction (the ~1us
    per-op gpsimd dispatch was a large serial cost at 64 ops).
  - edge-type one-hot masks and the -9e15 bias plane precomputed on host
    and DMA'd (kills the per-batch is_equal and neg-fill DVE ops).
  - all small elementwise ops grouped over 8 batches to amortize the
    fixed per-instruction engine overheads (DVE ~58cyc, ACT ~352cyc).
  - select-by-adj: w = m4 (.) e4 read directly from PSUM (the single
    pass over the 4-plane logits), collapsed by a 3-op add tree
    (disjoint one-hot planes + additive bias plane), then lrelu+exp on
    the collapsed [L, 8*L] group tile only.
  - per-batch softmax normalize moved to ACT (Copy with per-partition
    reciprocal scale); reciprocal stays on DVE.

Per-batch device algorithm (100=L nodes, D=128):
  h' = emb_aug[idx] -> [100, 132] bf16 (col 128 == 1.0)   (gather, x8)
  hT = h'.T[0:128]  -> PSUM, evac'd to SBUF bf16          (PE + ACT)
  scaled[:, b, k, :] = hT * a_k                           (DVE TS x4/group)
  e = hT.T @ scaled_b -> PSUM [100, 400]  e[j,k*100+i]=e_k[i,j]  (PE)
  w = m4 (.) e   (m4[j,k*100+i] = adj[i,j]==k+1, host)    (DVE, per pair)
  t = ((w0+w1)+(w2+w3)) + bias   [100, 8*100] group       (DVE x3)
  pT = exp(lrelu(t, 0.2))                                 (ACT x2, group)
  o = pT_b.T @ h' -> PSUM [100, 129]; col 128 = row sums  (PE)
  r = 1/o[:,128]; out = o[:,0:128]*r                      (DVE + ACT)
"""
import numpy as np

import concourse.bass as bass
import concourse.bacc as bacc
import concourse.tile as tile
from concourse import mybir
from concourse import bass_utils
from concourse.masks import make_identity

try:
    import ml_dtypes
    _BF16 = ml_dtypes.bfloat16
except ImportError:  # pragma: no cover
    import jax.numpy as jnp
    _BF16 = jnp.bfloat16

B, L, D, V = 512, 100, 128, 200000
NCORES = 8
BS = B // NCORES          # 64 batches per core
G = 8                     # batches per group (gather + elementwise grain)
NG = BS // G              # groups per core
DA = D + 4                # gathered row: 128 emb + ones col + 3 pad
NEG = -9e15
NEG_SLOPE = 0.2


def build_nc(reps: int = 1):
    """Build + compile the per-core Bass program (SPMD, shared by all cores).

    reps>1 wraps the whole 64-batch body in a hardware loop (for timing)."""
    nc = bacc.Bacc("TRN2", target_bir_lowering=False, debug=False,
                   enable_asserts=False, num_devices=NCORES)
    f32 = mybir.dt.float32
    bf16 = mybir.dt.bfloat16
    i32 = mybir.dt.int32

    emb = nc.dram_tensor("emb", [V, DA], bf16, kind="ExternalInput")
    idx_t = nc.dram_tensor("idx_t", [L, BS], i32, kind="ExternalInput")
    msk_t = nc.dram_tensor("msk_t", [L, BS, 4 * L], bf16, kind="ExternalInput")
    bias_t = nc.dram_tensor("bias_t", [L, BS, L], bf16, kind="ExternalInput")
    a_cols = nc.dram_tensor("a_cols", [D, 4], f32, kind="ExternalInput")
    out_d = nc.dram_tensor("out", [BS, L, D], f32, kind="ExternalOutput")

    from contextlib import ExitStack
    with tile.TileContext(nc) as tc, ExitStack() as ctx:
        cp = ctx.enter_context(tc.tile_pool(name="const", bufs=1))
        hp = ctx.enter_context(tc.tile_pool(name="hp", bufs=3))
        mp = ctx.enter_context(tc.tile_pool(name="mp", bufs=3))
        bp = ctx.enter_context(tc.tile_pool(name="bp", bufs=2))
        sb = ctx.enter_context(tc.tile_pool(name="sb", bufs=2))
        op = ctx.enter_context(tc.tile_pool(name="op", bufs=4))
        ps_hT = ctx.enter_context(tc.tile_pool(name="ps_hT", bufs=2,
                                               space="PSUM"))
        ps_e = ctx.enter_context(tc.tile_pool(name="ps_e", bufs=2,
                                              space="PSUM"))
        ps_o = ctx.enter_context(tc.tile_pool(name="ps_o", bufs=2,
                                              space="PSUM"))

        idx_sb = cp.tile([L, BS], i32)
        nc.sync.dma_start(out=idx_sb[:], in_=idx_t.ap())
        a_sb = cp.tile([D, 4], f32)
        nc.sync.dma_start(out=a_sb[:], in_=a_cols.ap())
        ident = cp.tile([L, L], bf16)
        make_identity(nc, ident[:])

        def body(_iv=None):
            for g in range(NG):
                b0 = g * G
                # ---- gather 8 batches in one indirect DMA ----
                h = hp.tile([L, G * DA], bf16, tag="h")
                for n in range(G):
                    nc.gpsimd.indirect_dma_start(
                        out=h[:, n * DA:(n + 1) * DA], out_offset=None,
                        in_=emb.ap(),
                        in_offset=bass.IndirectOffsetOnAxis(
                            ap=idx_sb[:, b0 + n:b0 + n + 1], axis=0))

                # ---- masks + bias for the group ----
                msk = mp.tile([L, G, 4 * L], bf16, tag="msk")
                nc.sync.dma_start(out=msk[:],
                                  in_=msk_t.ap()[:, b0:b0 + G, :])
                bias = bp.tile([L, G, L], bf16, tag="bias")
                nc.sync.dma_start(out=bias[:],
                                  in_=bias_t.ap()[:, b0:b0 + G, :])

                # ---- transposes (quad PSUM tiles) + evac to one group tile
                hT = sb.tile([D, G * L], bf16, tag="hT")
                for q in range(2):
                    hT_ps = ps_hT.tile([D, 4 * D], bf16, tag="hT_ps")
                    for n in range(4):
                        bb = q * 4 + n
                        nc.tensor.transpose(
                            out=hT_ps[:, n * D:n * D + L],
                            in_=h[:, bb * DA:bb * DA + D],
                            identity=ident[:])
                    nc.scalar.copy(
                        out=hT[:].rearrange("p (b i) -> p b i", b=G)
                            [:, q * 4:(q + 1) * 4, :],
                        in_=hT_ps[:].rearrange("p (b i) -> p b i", b=4)
                            [:, :, 0:L])

                # ---- scaled[:, b, k, :] = hT_b * a_k ----
                scaled = sb.tile([D, G * 4 * L], bf16, tag="scaled")
                sc_r = scaled[:].rearrange("p (b k i) -> p b k i", b=G, k=4)
                hT_r = hT[:].rearrange("p (b i) -> p b i", b=G)
                for k in range(4):
                    nc.vector.tensor_scalar(
                        out=sc_r[:, :, k, :], in0=hT_r,
                        scalar1=a_sb[:, k:k + 1], scalar2=None,
                        op0=mybir.AluOpType.mult)

                # ---- e matmuls (pair PSUM tiles) + masked select ----
                w = sb.tile([L, G * 4 * L], bf16, tag="w")
                w_r = w[:].rearrange("p (b x) -> p b x", b=G)
                for p in range(4):
                    e_ps = ps_e.tile([L, 2 * 512], f32, tag="e_ps")
                    for n in range(2):
                        bb = p * 2 + n
                        nc.tensor.matmul(
                            out=e_ps[:, n * 512:n * 512 + 4 * L],
                            lhsT=hT[:, bb * L:(bb + 1) * L],
                            rhs=scaled[:, bb * 4 * L:(bb + 1) * 4 * L],
                            start=True, stop=True)
                    nc.vector.tensor_tensor(
                        out=w_r[:, 2 * p:2 * p + 2, :],
                        in0=msk[:, 2 * p:2 * p + 2, :],
                        in1=e_ps[:].rearrange("p (n x) -> p n x", n=2)
                            [:, :, 0:4 * L],
                        op=mybir.AluOpType.mult)

                # ---- collapse planes: t = ((w0+w1)+(w2+w3)) + bias ----
                w4 = w[:].rearrange("p (b h x) -> p b h x", b=G, h=2)
                t1 = sb.tile([L, G * 2 * L], bf16, tag="t1")
                nc.vector.tensor_tensor(
                    out=t1[:].rearrange("p (b x) -> p b x", b=G),
                    in0=w4[:, :, 0, :], in1=w4[:, :, 1, :],
                    op=mybir.AluOpType.add)
                t14 = t1[:].rearrange("p (b h x) -> p b h x", b=G, h=2)
                t2 = sb.tile([L, G * L], bf16, tag="t2")
                nc.vector.tensor_tensor(
                    out=t2[:].rearrange("p (b x) -> p b x", b=G),
                    in0=t14[:, :, 0, :], in1=t14[:, :, 1, :],
                    op=mybir.AluOpType.add)
                t3 = sb.tile([L, G * L], bf16, tag="t3")
                nc.vector.tensor_tensor(
                    out=t3[:].rearrange("p (b x) -> p b x", b=G),
                    in0=t2[:].rearrange("p (b x) -> p b x", b=G),
                    in1=bias[:], op=mybir.AluOpType.add)

                # ---- pT = exp(lrelu(t)) on the whole group ----
                u = sb.tile([L, G * L], bf16, tag="u")
                nc.scalar.activation(out=u[:], in_=t3[:],
                                     func=mybir.ActivationFunctionType.Lrelu,
                                     alpha=NEG_SLOPE)
                pT = sb.tile([L, G * L], bf16, tag="pT")
                nc.scalar.activation(out=pT[:], in_=u[:],
                                     func=mybir.ActivationFunctionType.Exp)

                # ---- output matmuls + normalize (per-batch PSUM tiles) ----
                for n in range(G):
                    bb = b0 + n
                    o_ps = ps_o.tile([L, D + 1], f32, tag="o_ps")
                    nc.tensor.matmul(
                        out=o_ps[:],
                        lhsT=pT[:, n * L:(n + 1) * L],
                        rhs=h[:, n * DA:n * DA + D + 1],
                        start=True, stop=True)
                    r = op.tile([L, 1], f32, tag="r")
                    nc.vector.reciprocal(r[:], o_ps[:, D:D + 1])
                    o_sb = op.tile([L, D], f32, tag="o_sb")
                    nc.scalar.activation(out=o_sb[:], in_=o_ps[:, 0:D],
                                         func=mybir.ActivationFunctionType.Copy,
                                         scale=r[:, 0:1])
                    nc.sync.dma_start(out=out_d.ap()[bb], in_=o_sb[:])

        if reps == 1:
            body()
        else:
            with tc.For_i(0, reps, 1) as iv:
                body(iv)

    nc.compile()
    return nc


_CACHED_NC = None


def _shard_inputs(inputs, adj, emb_table, a0, a1, a2, a3):
    inputs = np.asarray(inputs).astype(np.int32)
    adj = np.asarray(adj)
    emb_table = np.asarray(emb_table, dtype=np.float32)
    avecs = [np.asarray(a, dtype=np.float32) for a in (a0, a1, a2, a3)]

    emb_aug = np.concatenate(
        [emb_table, np.ones((V, 1), np.float32), np.zeros((V, 3), np.float32)],
        axis=1).astype(_BF16)                                   # [V, 132]
    a_cols = np.stack(avecs, axis=1)                            # [128, 4]

    in_maps = []
    for c in range(NCORES):
        sl = slice(c * BS, (c + 1) * BS)
        idx_c = np.ascontiguousarray(inputs[sl].T)              # [L, BS]
        adjT = adj[sl].transpose(2, 0, 1)                       # [j, b, i]
        msk = np.stack([(adjT == k + 1) for k in range(4)],
                       axis=2)                                  # [j, b, 4, i]
        msk = np.ascontiguousarray(msk).astype(_BF16).reshape(L, BS, 4 * L)
        bias = np.where(adjT == 0, np.float32(NEG),
                        np.float32(0.0)).astype(_BF16)          # [j, b, i]
        bias = np.ascontiguousarray(bias)
        in_maps.append(dict(emb=emb_aug, idx_t=idx_c, msk_t=msk,
                            bias_t=bias, a_cols=a_cols))
    return in_maps


def kernel(inputs, adj, mask_item, item, emb_table, a0, a1, a2, a3):
    """Full inputs in, full output out. mask_item/item are unused by the
    reference model's forward pass."""
    global _CACHED_NC
    if _CACHED_NC is None:
        _CACHED_NC = build_nc(reps=1)
    nc = _CACHED_NC

    in_maps = _shard_inputs(inputs, adj, emb_table, a0, a1, a2, a3)
    res = bass_utils.run_bass_kernel_spmd(nc, in_maps,
                                          core_ids=list(range(NCORES)))
    out = np.concatenate([np.asarray(res.results[c]["out"])
                          for c in range(NCORES)], axis=0)
    return out


# revision 7
# speedup vs baseline: 1.8670x; 1.8670x over previous
"""Trainium2 Bass kernel for nn_CombineGraph (GCE-GNN LocalAggregator), v2.

Computation (per batch b):
    h = emb_table[inputs[b]]                         # [L, D]
    e_k[i,j] = leakyrelu(sum_d h[i,d]*h[j,d]*a_k[d]) # 4 edge-type logits
    alpha = softmax_j(select-by-adj(e_k), -9e15 fill)
    out[b] = alpha @ h

Sharding: pure data-parallel over batch B=512 across 8 NeuronCores
(64 batches/core). emb table + a-vectors replicated; no collectives.

v2 changes vs v1 (250us -> target ~65us):
  - bf16 everywhere on the PE path (4x matmul throughput) and on DVE
    elementwise ops (2-4x modes).
  - indirect gathers batched 8 batches per SWDGE instruction (the ~1us
    per-op gpsimd dispatch was a large serial cost at 64 ops).
  - edge-type one-hot masks and the -9e15 bias plane precomputed on host
    and DMA'd (kills the per-batch is_equal and neg-fill DVE ops).
  - all small elementwise ops grouped over 8 batches to amortize the
    fixed per-instruction engine overheads (DVE ~58cyc, ACT ~352cyc).
  - select-by-adj: w = m4 (.) e4 read directly from PSUM (the single
    pass over the 4-plane logits), collapsed by a 3-op add tree
    (disjoint one-hot planes + additive bias plane), then lrelu+exp on
    the collapsed [L, 8*L] group tile only.
  - per-batch softmax normalize on DVE (reciprocal + tensor_scalar);
    the ACT Copy+scale variant measured 2x slower (per-batch DVE->ACT
    serialization).

Per-batch device algorithm (100=L nodes, D=128):
  h' = emb_aug[idx] -> [100, 132] bf16 (col 128 == 1.0)   (gather, x8)
  hT = h'.T[0:128]  -> PSUM, evac'd to SBUF bf16          (PE + ACT)
  scaled[:, b, k, :] = hT * a_k                           (DVE TS x4/group)
  e = hT.T @ scaled_b -> PSUM [100, 400]  e[j,k*100+i]=e_k[i,j]  (PE)
  w = m4 (.) e   (m4[j,k*100+i] = adj[i,j]==k+1, host)    (DVE, per pair)
  t = ((w0+w1)+(w2+w3)) + bias   [100, 8*100] group       (DVE x3)
  pT = exp(lrelu(t, 0.2))                                 (ACT x2, group)
  o = pT_b.T @ h' -> PSUM [100, 129]; col 128 = row sums  (PE)
  r = 1/o[:,128]; out = o[:,0:128]*r                      (DVE + ACT)
"""
import numpy as np

import concourse.bass as bass
import concourse.bacc as bacc
import concourse.tile as tile
from concourse import mybir
from concourse import bass_utils
from concourse.masks import make_identity

try:
    import ml_dtypes
    _BF16 = ml_dtypes.bfloat16
except ImportError:  # pragma: no cover
    import jax.numpy as jnp
    _BF16 = jnp.bfloat16

B, L, D, V = 512, 100, 128, 200000
NCORES = 8
BS = B // NCORES          # 64 batches per core
G = 8                     # batches per group (gather + elementwise grain)
NG = BS // G              # groups per core
DA = D + 4                # gathered row: 128 emb + ones col + 3 pad
NEG = -9e15
NEG_SLOPE = 0.2


def build_nc(reps: int = 1):
    """Build + compile the per-core Bass program (SPMD, shared by all cores).

    reps>1 wraps the whole 64-batch body in a hardware loop (for timing)."""
    nc = bacc.Bacc("TRN2", target_bir_lowering=False, debug=False,
                   enable_asserts=False, num_devices=NCORES)
    f32 = mybir.dt.float32
    bf16 = mybir.dt.bfloat16
    i32 = mybir.dt.int32

    emb = nc.dram_tensor("emb", [V, DA], bf16, kind="ExternalInput")
    idx_t = nc.dram_tensor("idx_t", [L, BS], i32, kind="ExternalInput")
    msk_t = nc.dram_tensor("msk_t", [L, BS, 4 * L], bf16, kind="ExternalInput")
    bias_t = nc.dram_tensor("bias_t", [L, BS, L], bf16, kind="ExternalInput")
    a_cols = nc.dram_tensor("a_cols", [D, 4], f32, kind="ExternalInput")
    out_d = nc.dram_tensor("out", [BS, L, D], f32, kind="ExternalOutput")

    from contextlib import ExitStack
    with tile.TileContext(nc) as tc, ExitStack() as ctx:
        cp = ctx.enter_context(tc.tile_pool(name="const", bufs=1))
        hp = ctx.enter_context(tc.tile_pool(name="hp", bufs=3))
        mp = ctx.enter_context(tc.tile_pool(name="mp", bufs=3))
        bp = ctx.enter_context(tc.tile_pool(name="bp", bufs=2))
        sb = ctx.enter_context(tc.tile_pool(name="sb", bufs=2))
        op = ctx.enter_context(tc.tile_pool(name="op", bufs=4))
        ps_hT = ctx.enter_context(tc.tile_pool(name="ps_hT", bufs=2,
                                               space="PSUM"))
        ps_e = ctx.enter_context(tc.tile_pool(name="ps_e", bufs=2,
                                              space="PSUM"))
        ps_o = ctx.enter_context(tc.tile_pool(name="ps_o", bufs=2,
                                              space="PSUM"))

        idx_sb = cp.tile([L, BS], i32)
        nc.sync.dma_start(out=idx_sb[:], in_=idx_t.ap())
        a_sb = cp.tile([D, 4], f32)
        nc.sync.dma_start(out=a_sb[:], in_=a_cols.ap())
        ident = cp.tile([L, L], bf16)
        make_identity(nc, ident[:])

        def body(_iv=None):
            for g in range(NG):
                b0 = g * G
                # ---- gather 8 batches in one indirect DMA ----
                h = hp.tile([L, G * DA], bf16, tag="h")
                for n in range(G):
                    nc.gpsimd.indirect_dma_start(
                        out=h[:, n * DA:(n + 1) * DA], out_offset=None,
                        in_=emb.ap(),
                        in_offset=bass.IndirectOffsetOnAxis(
                            ap=idx_sb[:, b0 + n:b0 + n + 1], axis=0))

                # ---- masks + bias for the group ----
                msk = mp.tile([L, G, 4 * L], bf16, tag="msk")
                nc.sync.dma_start(out=msk[:],
                                  in_=msk_t.ap()[:, b0:b0 + G, :])
                bias = bp.tile([L, G, L], bf16, tag="bias")
                nc.sync.dma_start(out=bias[:],
                                  in_=bias_t.ap()[:, b0:b0 + G, :])

                # ---- transposes (quad PSUM tiles) + evac to one group tile
                hT = sb.tile([D, G * L], bf16, tag="hT")
                for q in range(2):
                    hT_ps = ps_hT.tile([D, 4 * D], bf16, tag="hT_ps")
                    for n in range(4):
                        bb = q * 4 + n
                        nc.tensor.transpose(
                            out=hT_ps[:, n * D:n * D + L],
                            in_=h[:, bb * DA:bb * DA + D],
                            identity=ident[:])
                    nc.scalar.copy(
                        out=hT[:].rearrange("p (b i) -> p b i", b=G)
                            [:, q * 4:(q + 1) * 4, :],
                        in_=hT_ps[:].rearrange("p (b i) -> p b i", b=4)
                            [:, :, 0:L])

                # ---- scaled[:, b, k, :] = hT_b * a_k ----
                scaled = sb.tile([D, G * 4 * L], bf16, tag="scaled")
                sc_r = scaled[:].rearrange("p (b k i) -> p b k i", b=G, k=4)
                hT_r = hT[:].rearrange("p (b i) -> p b i", b=G)
                for k in range(4):
                    nc.vector.tensor_scalar(
                        out=sc_r[:, :, k, :], in0=hT_r,
                        scalar1=a_sb[:, k:k + 1], scalar2=None,
                        op0=mybir.AluOpType.mult)

                # ---- e matmuls (pair PSUM tiles) + masked select ----
                w = sb.tile([L, G * 4 * L], bf16, tag="w")
                w_r = w[:].rearrange("p (b x) -> p b x", b=G)
                for p in range(4):
                    e_ps = ps_e.tile([L, 2 * 512], f32, tag="e_ps")
                    for n in range(2):
                        bb = p * 2 + n
                        nc.tensor.matmul(
                            out=e_ps[:, n * 512:n * 512 + 4 * L],
                            lhsT=hT[:, bb * L:(bb + 1) * L],
                            rhs=scaled[:, bb * 4 * L:(bb + 1) * 4 * L],
                            start=True, stop=True)
                    nc.vector.tensor_tensor(
                        out=w_r[:, 2 * p:2 * p + 2, :],
                        in0=msk[:, 2 * p:2 * p + 2, :],
                        in1=e_ps[:].rearrange("p (n x) -> p n x", n=2)
                            [:, :, 0:4 * L],
                        op=mybir.AluOpType.mult)

                # ---- collapse planes: t = ((w0+w1)+(w2+w3)) + bias ----
                w4 = w[:].rearrange("p (b h x) -> p b h x", b=G, h=2)
                t1 = sb.tile([L, G * 2 * L], bf16, tag="t1")
                nc.vector.tensor_tensor(
                    out=t1[:].rearrange("p (b x) -> p b x", b=G),
                    in0=w4[:, :, 0, :], in1=w4[:, :, 1, :],
                    op=mybir.AluOpType.add)
                t14 = t1[:].rearrange("p (b h x) -> p b h x", b=G, h=2)
                t2 = sb.tile([L, G * L], bf16, tag="t2")
                nc.vector.tensor_tensor(
                    out=t2[:].rearrange("p (b x) -> p b x", b=G),
                    in0=t14[:, :, 0, :], in1=t14[:, :, 1, :],
                    op=mybir.AluOpType.add)
                t3 = sb.tile([L, G * L], bf16, tag="t3")
                nc.vector.tensor_tensor(
                    out=t3[:].rearrange("p (b x) -> p b x", b=G),
                    in0=t2[:].rearrange("p (b x) -> p b x", b=G),
                    in1=bias[:], op=mybir.AluOpType.add)

                # ---- pT = exp(lrelu(t)) on the whole group ----
                u = sb.tile([L, G * L], bf16, tag="u")
                nc.scalar.activation(out=u[:], in_=t3[:],
                                     func=mybir.ActivationFunctionType.Lrelu,
                                     alpha=NEG_SLOPE)
                pT = sb.tile([L, G * L], bf16, tag="pT")
                nc.scalar.activation(out=pT[:], in_=u[:],
                                     func=mybir.ActivationFunctionType.Exp)

                # ---- output matmuls + normalize (per-batch PSUM tiles) ----
                for n in range(G):
                    bb = b0 + n
                    o_ps = ps_o.tile([L, D + 1], f32, tag="o_ps")
                    nc.tensor.matmul(
                        out=o_ps[:],
                        lhsT=pT[:, n * L:(n + 1) * L],
                        rhs=h[:, n * DA:n * DA + D + 1],
                        start=True, stop=True)
                    r = op.tile([L, 1], f32, tag="r")
                    nc.vector.reciprocal(r[:], o_ps[:, D:D + 1])
                    o_sb = op.tile([L, D], f32, tag="o_sb")
                    nc.vector.tensor_scalar(out=o_sb[:], in0=o_ps[:, 0:D],
                                            scalar1=r[:, 0:1], scalar2=None,
                                            op0=mybir.AluOpType.mult)
                    nc.sync.dma_start(out=out_d.ap()[bb], in_=o_sb[:])

        if reps == 1:
            body()
        else:
            with tc.For_i(0, reps, 1) as iv:
                body(iv)

    nc.compile()
    return nc


_CACHED_NC = None


def _shard_inputs(inputs, adj, emb_table, a0, a1, a2, a3):
    inputs = np.asarray(inputs).astype(np.int32)
    adj = np.asarray(adj)
    emb_table = np.asarray(emb_table, dtype=np.float32)
    avecs = [np.asarray(a, dtype=np.float32) for a in (a0, a1, a2, a3)]

    emb_aug = np.concatenate(
        [emb_table, np.ones((V, 1), np.float32), np.zeros((V, 3), np.float32)],
        axis=1).astype(_BF16)                                   # [V, 132]
    a_cols = np.stack(avecs, axis=1)                            # [128, 4]

    in_maps = []
    for c in range(NCORES):
        sl = slice(c * BS, (c + 1) * BS)
        idx_c = np.ascontiguousarray(inputs[sl].T)              # [L, BS]
        adjT = adj[sl].transpose(2, 0, 1)                       # [j, b, i]
        msk = np.stack([(adjT == k + 1) for k in range(4)],
                       axis=2)                                  # [j, b, 4, i]
        msk = np.ascontiguousarray(msk).astype(_BF16).reshape(L, BS, 4 * L)
        bias = np.where(adjT == 0, np.float32(NEG),
                        np.float32(0.0)).astype(_BF16)          # [j, b, i]
        bias = np.ascontiguousarray(bias)
        in_maps.append(dict(emb=emb_aug, idx_t=idx_c, msk_t=msk,
                            bias_t=bias, a_cols=a_cols))
    return in_maps


def kernel(inputs, adj, mask_item, item, emb_table, a0, a1, a2, a3):
    """Full inputs in, full output out. mask_item/item are unused by the
    reference model's forward pass."""
    global _CACHED_NC
    if _CACHED_NC is None:
        _CACHED_NC = build_nc(reps=1)
    nc = _CACHED_NC

    in_maps = _shard_inputs(inputs, adj, emb_table, a0, a1, a2, a3)
    res = bass_utils.run_bass_kernel_spmd(nc, in_maps,
                                          core_ids=list(range(NCORES)))
    out = np.concatenate([np.asarray(res.results[c]["out"])
                          for c in range(NCORES)], axis=0)
    return out
